# revision 64
# baseline (speedup 1.0000x reference)
"""Self-contained Trainium2 Bass kernel for nn_MultiHeadAttention_80599356276988.

Strategy (v4): tensor-parallel over heads (2 heads/core x 8 cores).
Numerics as v2 (fp8 only for Q/K projections, where exp squashes the
quantization error; everything else bf16). Structured for overlap:
  - X loads as 8 consolidated DMA instructions (DMA ring issue time is
    ~600ns per instruction; 64 small DMAs would cost ~38us of ring time).
  - warmup AllToAll at t=0 absorbs the one-time CC barrier; gpsimd queue
    carries ONLY collectives, so their queue-blocking cannot stall
    compute (masks run on DVE, the norm broadcast on the PE).
  - softmax norm: denominator row -> reciprocal (DVE) -> fp16 rank-1
    PE matmul broadcast -> DVE multiply; pav is copied out of PSUM
    immediately so the single psum accumulator frees for the next group.
  - attention inner loop software-pipelined: scores(st+1) on the PE
    before AV(st), hiding the exp latency.
  - QKV and output-projection matmuls queued as background closures
    drained into PE bubbles between stiles; per-group projection starts
    right after its AllToAll.
"""
import sys

sys.path.insert(0, "/opt/trn_rl_repo")
import numpy as np
import ml_dtypes
from collections import deque
from contextlib import ExitStack

import concourse.bass as bass
import concourse.mybir as mybir
import concourse.tile as tile
from concourse import bacc
from concourse.bass_utils import run_bass_kernel_spmd
F32 = mybir.dt.float32
BF16 = mybir.dt.bfloat16
FP16 = mybir.dt.float16
FP8 = mybir.dt.float8e4
DR = mybir.MatmulPerfMode.DoubleRow
EXP = mybir.ActivationFunctionType.Exp

B, T, C = 2, 2048, 1024
H, D = 16, 64
NCORES = 8
HPC = H // NCORES        # heads per core = 2
N = B * T                # 4096 flat rows
RPB = T // NCORES        # rows per core per batch = 256
SCALE = float(C) ** -0.5 / 1024.0  # /32^2: Wq,Wk prescaled x32 for fp8

_CACHE = {}


def build_nc():
    nc = bacc.Bacc(num_devices=NCORES)

    XT8 = nc.dram_tensor("xt8", [128, 8 * N], BF16, kind="ExternalInput")
    XQ8 = nc.dram_tensor("xq8", [128, 8 * N], FP8, kind="ExternalInput")
    WQ8 = nc.dram_tensor("wq8", [128, C], FP8, kind="ExternalInput")
    WK8 = nc.dram_tensor("wk8", [128, C], FP8, kind="ExternalInput")
    WV3 = nc.dram_tensor("wv3", [128, C], BF16, kind="ExternalInput")
    WPT = nc.dram_tensor("wpt8", [C, C], BF16, kind="ExternalInput")
    BIASB = nc.dram_tensor("biasbf", [1, C], BF16, kind="ExternalInput")
    MASKC = nc.dram_tensor("maskc", [128, 128], BF16, kind="ExternalInput")
    ZER = nc.dram_tensor("zeros", [64, N], BF16, kind="ExternalInput")
    IDENT = nc.dram_tensor("ident", [128, 64], BF16, kind="ExternalInput")
    OUT = nc.dram_tensor("out", [2 * RPB, C], F32, kind="ExternalOutput")

    # slot granularity (b, qh): each rank gets 128 rows of each query-half.
    # One exchange per (group, head) so the last head's AllToAll is only
    # 128KB and the first head's overlaps the second head's compute.
    a2a_in = [[nc.dram_tensor(f"a2a_in{g}_{h}", [NCORES, 64, 128], BF16)
               for h in range(2)] for g in range(4)]
    a2a_out = [[nc.dram_tensor(f"a2a_out{g}_{h}", [NCORES, 64, 128], BF16)
                for h in range(2)] for g in range(4)]
    warm_in = nc.dram_tensor("warm_in", [NCORES, 512], BF16)
    warm_out = nc.dram_tensor("warm_out", [NCORES, 512], BF16)


    XQv = XQ8[:].rearrange("p (c n) -> p c n", c=8)
    XTv = XT8[:].rearrange("p (c n) -> p c n", c=8)

    with tile.TileContext(nc) as tc, ExitStack() as ctx:
        consts = ctx.enter_context(tc.tile_pool(name="consts", bufs=1))
        qkv = ctx.enter_context(tc.tile_pool(name="qkv", bufs=1))
        xtp = ctx.enter_context(tc.tile_pool(name="xtp", bufs=1))
        vqp = ctx.enter_context(tc.tile_pool(name="vqp", bufs=2))
        psr = ctx.enter_context(tc.tile_pool(name="psr", bufs=1, space="PSUM"))
        pss = ctx.enter_context(tc.tile_pool(name="pss", bufs=2, space="PSUM"))
        pop = ctx.enter_context(tc.tile_pool(name="pop", bufs=1, space="PSUM"))
        pavp = ctx.enter_context(tc.tile_pool(name="pavp", bufs=1,
                                              space="PSUM"))
        pgp = ctx.enter_context(tc.tile_pool(name="pgp", bufs=4))
        nrm = ctx.enter_context(tc.tile_pool(name="nrm", bufs=2))
        rvp = ctx.enter_context(tc.tile_pool(name="rvp", bufs=2))
        dp = ctx.enter_context(tc.tile_pool(name="dp", bufs=2))

        # ---- background PE work queue ----
        bg = deque()

        def drain_bg(k):
            for _ in range(k):
                if not bg:
                    return
                bg.popleft()()

        # ---- constants ----
        wq_sb = consts.tile([128, 8, 128], FP8, tag="wq")
        wk_sb = consts.tile([128, 8, 128], FP8, tag="wk")
        wv_sb = consts.tile([128, C], BF16, tag="wv")
        nc.sync.dma_start(wq_sb[:].rearrange("p c m -> p (c m)"), WQ8[:])
        nc.sync.dma_start(wk_sb[:].rearrange("p c m -> p (c m)"), WK8[:])
        nc.sync.dma_start(wv_sb[:], WV3[:])
        ones1 = consts.tile([1, 128], BF16, tag="ones1")
        nc.vector.memset(ones1[:], 1.0)
        ones16 = consts.tile([1, 64], FP16, tag="ones16")
        nc.vector.memset(ones16[:], 1.0)
        ident = consts.tile([128, 64], BF16, tag="ident")
        nc.sync.dma_start(ident[:], IDENT[:])

        # warmup collective FIRST on gpsimd (nothing else ever runs
        # there): absorbs the one-time CC-stream barrier while phase A
        # loads/computes.
        nc.sync.dma_start(warm_in[:], ZER[0:8, 0:512])
        nc.gpsimd.collective_compute(
            "AllToAll", mybir.AluOpType.bypass,
            replica_groups=[list(range(NCORES))],
            ins=[warm_in[:]], outs=[warm_out[:]])

        # ---- persistent activations ----
        QT = qkv.tile([128, N], BF16, tag="QT")
        # KTz: [128, 2N]; head h block at cols h*N + global token. Rows of
        # the other head are zeroed so scores contract over the full 128.
        KTz = qkv.tile([128, 2 * N], BF16, tag="KTz")
        # VS: per (b,h) 16 key-stiles of [128 keys, 64 dims + 1 ones col]
        VS = qkv.tile([128, 4 * 16 * 128], BF16, tag="VS")
        bias_bf = consts.tile([1, C], BF16, tag="bias")
        mask128 = consts.tile([128, 128], BF16, tag="mask128")
        for b in range(B):
            for h in range(HPC):
                vb = (b * HPC + h) * 2048
                nc.vector.memset(VS[:, vb + 64:vb + 2048:128], 1.0)

        # ---------------- Phase A: QKV projections ----------------
        # X loads: one consolidated DMA per (quarter, tensor). Quarters
        # 0,1 upfront on the sync ring; 2,3 emitted later on the scalar
        # ring (mid-schedule) so their buffer-reuse waits can't
        # head-of-line-block the early exps on the scalar engine.
        xts_all, xq_all = {}, {}

        def phase_a_dmas(qr, eng, split=False):
            xq_t = xtp.tile([128, 8, 1024], FP8, tag="xq", bufs=2,
                            name=f"xq{qr}")
            xt_t = xtp.tile([128, 8, 1024], BF16, tag="xt", bufs=2,
                            name=f"xt{qr}")
            lo, hi = qr * 1024, (qr + 1) * 1024
            if split:
                eng.dma_start(xq_t[:, 0:4, :], XQv[:, 0:4, lo:hi])
                eng.dma_start(xq_t[:, 4:8, :], XQv[:, 4:8, lo:hi])
            else:
                eng.dma_start(xq_t[:], XQv[:, :, lo:hi])
                eng.dma_start(xt_t[:], XTv[:, :, lo:hi])
            xq_all[qr], xts_all[qr] = xq_t, xt_t

        # Serial sync-ring order tuned to consumption order: q0's fp8
        # first (q/k DR matmuls), head-0 zeros before the first scores,
        # xt0 chunks for the inline v0 chains, then q1's fp8 (the first
        # background drains), head-1 zeros, xt1.
        phase_a_dmas(0, nc.sync, split=True)
        xt0 = xts_all[0]
        nc.sync.dma_start(KTz[64:128, 0:N], ZER[:])
        nc.sync.dma_start(xt0[:, 0:4, :], XTv[:, 0:4, 0:1024])
        nc.sync.dma_start(xt0[:, 4:8, :], XTv[:, 4:8, 0:1024])
        phase_a_dmas(1, nc.sync, split=True)
        nc.sync.dma_start(KTz[0:64, N:2 * N], ZER[:])
        xt1 = xts_all[1]
        nc.sync.dma_start(xt1[:, 0:4, :], XTv[:, 0:4, 1024:2048])
        nc.sync.dma_start(xt1[:, 4:8, :], XTv[:, 4:8, 1024:2048])
        nc.sync.dma_start(bias_bf[:], BIASB[:])
        nc.sync.dma_start(mask128[:], MASKC[:])

        # output-projection weights late on the scalar ring (needed ~100us)
        wpt_sb = [consts.tile([128, C], BF16, tag=f"wpt{j}", name=f"wpt{j}")
                  for j in range(8)]

        def load_wpt():
            for j in range(8):
                nc.scalar.dma_start(wpt_sb[j][:], WPT[j * 128:(j + 1) * 128, :])

        vq_all = {}

        def phase_a_qk_closures(qr):
            """PE closures: q,k projections of quarter qr (fp8 DR)."""
            xq_t = xq_all[qr]

            def make_qk(w_sb, nm):
                for hf in range(2):
                    pp = psr.tile([128, 512], F32, tag="pp",
                                  name=f"pp_{nm}{qr}_{hf}")
                    def mm(pp, hf, cc):
                        def go():
                            nc.tensor.matmul(
                                pp[:],
                                w_sb[:, cc:cc + 2, :],
                                xq_t[:, cc:cc + 2, hf * 512:(hf + 1) * 512],
                                perf_mode=DR,
                                start=(cc == 0), stop=(cc == 6))
                        return go
                    for cc in range(0, 8, 2):
                        yield mm(pp, hf, cc)
                    lo = qr * 1024 + hf * 512
                    if nm == "q":
                        def cp(pp, lo):
                            def go():
                                nc.vector.tensor_copy(
                                    QT[:, lo:lo + 512], pp[:])
                            return go
                        yield cp(pp, lo)
                    else:
                        def cpk(pp, lo):
                            def go():
                                nc.vector.tensor_copy(
                                    KTz[0:64, lo:lo + 512], pp[0:64, :])
                                nc.vector.tensor_copy(
                                    KTz[64:128, N + lo:N + lo + 512],
                                    pp[64:128, :])
                            return go
                        yield cpk(pp, lo)

            yield from make_qk(wq_sb, "q")
            yield from make_qk(wk_sb, "k")

        def phase_a_v_closures(qr):
            """PE closures: v projection + VS transpose of quarter qr.

            ALL transposes must be EMITTED before any collective: Tile
            serializes XBAR dma-transposes against collectives (shared
            hardware), in emission order.
            """
            b, qhh = qr // 2, qr % 2
            xt_t = xts_all[qr]
            vq_t = vqp.tile([128, 1024], BF16, tag="vq", name=f"vq{qr}")
            vq_all[qr] = vq_t

            for hf in range(2):
                pp = psr.tile([128, 512], F32, tag="pp", name=f"pp_v{qr}_{hf}")
                def mmv(pp, hf, cc):
                    def go():
                        nc.tensor.matmul(
                            pp[:],
                            wv_sb[:, cc * 128:(cc + 1) * 128],
                            xt_t[:, cc, hf * 512:(hf + 1) * 512],
                            start=(cc == 0), stop=(cc == 7))
                    return go
                for cc in range(8):
                    yield mmv(pp, hf, cc)
                def cpv(pp, hf):
                    def go():
                        nc.scalar.copy(vq_t[:, hf * 512:(hf + 1) * 512], pp[:])
                    return go
                yield cpv(pp, hf)

            # V transpose on the PE (identity matmul): XBAR dma-transposes
            # are serialized against collectives by the framework and the
            # scheduler interleaves them unpredictably; the PE path has
            # no such coupling. [64,128] chunk -> [128,64] psum (f32),
            # 8 chunks batched per head, one strided DVE copy to VS.
            for h in range(HPC):
                vb = (b * HPC + h) * 2048
                tp = psr.tile([128, 8, 64], F32, tag="pp",
                              name=f"tp{qr}_{h}")
                def mmt(tp, h, c0):
                    def go():
                        for ch in range(c0, c0 + 4):
                            nc.tensor.matmul(
                                tp[:, ch, :],
                                vq_t[h * 64:(h + 1) * 64,
                                     ch * 128:(ch + 1) * 128],
                                ident[h * 64:(h + 1) * 64, :],
                                start=True, stop=True,
                                skip_group_check=True)
                    return go
                yield mmt(tp, h, 0)
                yield mmt(tp, h, 4)
                def cpt(tp, h, vb):
                    def go():
                        vs3 = VS[:, vb + qhh * 8 * 128:
                                 vb + (qhh + 1) * 8 * 128]\
                            .rearrange("p (s c) -> p s c", s=8)
                        nc.vector.tensor_copy(vs3[:, :, 0:64], tp[:])
                    return go
                yield cpt(tp, h, vb)

        # ---------------- Phase B: attention ----------------
        # fast-exp on DVE (Schraudolph, bf16-bits): P = 2^(x*SCALE*log2e)
        # via one mult+add with int16 convert, bitcast to bf16. ~1.5%
        # relative error, zero-mean; softmax normalization cancels the
        # correlated part. Used on alternate stiles of late groups to
        # relieve the scalar engine.
        FEXP_K1 = SCALE * 1.4426950408889634 * 128.0
        FEXP_K2 = 16256.0 - 5.5
        I16 = mybir.dt.int16

        def attn_group(b, h, qh, drain=2, inject=None, fexp=False,
                       last=False):
            """1024 queries [qh*1024,(qh+1)*1024) of batch b, head h.

            `inject`: {stile: [closures]} emitted at that stile (e.g.
            the previous group's norm finish at stile 3, by which time
            its DVE inputs are ready).
            """
            inject = inject or {}
            nst = 8 * (qh + 1)
            vb = (b * HPC + h) * 2048
            kcol = h * N + b * T
            qbase = b * T + qh * 1024
            pav = pavp.tile([65, 1024], F32, tag="pav", name=f"pav{b}{h}{qh}")

            def emit_av(avs):
                for lo, hi, st, Pg in avs:
                    last = (8 * qh + 3) if hi == 512 else (nst - 1)
                    nc.tensor.matmul(
                        pav[0:65, lo:hi],
                        VS[:, vb + st * 128:vb + st * 128 + 65],
                        Pg[:, lo:hi],
                        start=(st == 0), stop=(st == last),
                        skip_group_check=True)

            pending = None
            for st in range(nst):
                off = max(0, st * 128 - qh * 1024)
                halves = [(max(off, 512 * hf), 512 * (hf + 1))
                          for hf in range(2) if off < 512 * (hf + 1)]
                Pg = pgp.tile([128, 1024], BF16, tag="Pg",
                              name=f"Pg{b}{h}{qh}_{st}")
                ps = pss.tile([128, 1024], F32, tag="ps",
                              name=f"ps{b}{h}{qh}_{st}")
                for lo, hi in halves:
                    nc.tensor.matmul(
                        ps[:, lo:hi],
                        KTz[:, kcol + st * 128:kcol + (st + 1) * 128],
                        QT[:, qbase + lo:qbase + hi],
                        start=True, stop=True)
                if fexp and st % 2 == 1:
                    nc.vector.tensor_scalar(
                        Pg[:, off:1024].bitcast(I16), ps[:, off:1024],
                        FEXP_K1, FEXP_K2,
                        mybir.AluOpType.mult, mybir.AluOpType.add)
                else:
                    nc.scalar.activation(Pg[:, off:1024], ps[:, off:1024],
                                         EXP, scale=SCALE)
                if st >= 8 * qh:
                    # causal mask inside the diagonal 128-col block only
                    nc.vector.tensor_mul(
                        Pg[:, off:off + 128], Pg[:, off:off + 128],
                        mask128[:])
                if pending is not None:
                    emit_av(pending)
                for cl in inject.get(st, ()):
                    cl()
                drain_bg(drain)
                pending = [(lo, hi, st, Pg) for lo, hi in halves]
            emit_av(pending)

            # ---- norm, phase 1 (inline, DVE only): free the pav slot ----
            # The last group skips the pav->sbuf bounce (no next group
            # needs the psum slot; shortens the tail-critical chain).
            g = b * 2 + qh
            pav_sb = nrm.tile([64, 1024], F32, tag="pav_sb",
                              name=f"pav_sb{b}{h}{qh}")
            nc.vector.tensor_copy(pav_sb[:], pav[0:64, :])
            den0 = nrm.tile([1, 1024], F32, tag="den0", name=f"den{b}{h}{qh}")
            nc.vector.tensor_copy(den0[:], pav[64:65, :])
            rcp = nrm.tile([1, 1024], F32, tag="rcp", name=f"rcp{b}{h}{qh}")
            nc.vector.reciprocal_approx_fast(out=rcp[:], in_=den0[:])
            rcp16 = nrm.tile([1, 1024], FP16, tag="rcp16",
                             name=f"rcp16{b}{h}{qh}")
            nc.vector.tensor_copy(rcp16[:], rcp[:])

            # ---- norm, phase 2 (late closure: PE broadcast + multiply) ----
            def norm_fin():
                rb = pss.tile([128, 1024], F32, tag="ps",
                              name=f"rb{b}{h}{qh}")
                for lo in (0, 512):
                    nc.tensor.matmul(
                        rb[0:64, lo:lo + 512], ones16[:],
                        rcp16[:, lo:lo + 512],
                        start=True, stop=True)
                act_t = nrm.tile([64, 1024], BF16, tag="act",
                                 name=f"act{b}{h}{qh}")
                nc.vector.tensor_mul(act_t[:], pav_sb[:], rb[0:64, :])
                nc.sync.dma_start(
                    a2a_in[g][h][:].rearrange("j d r -> d j r"), act_t[:])
            return norm_fin

        def coll(g, h):
            nc.gpsimd.collective_compute(
                "AllToAll", mybir.AluOpType.bypass,
                replica_groups=[list(range(NCORES))],
                ins=[a2a_in[g][h][:]], outs=[a2a_out[g][h][:]])

        # ---------------- Phase D: output projection ----------------
        rv_all = {}

        def proj_dma(g, heads=(0, 1)):
            if g in rv_all:
                rv = rv_all[g]
            else:
                rv = rvp.tile([128, 8 * 128], BF16, tag="rv", name=f"rv{g}")
                rv_all[g] = rv
            for h in heads:
                nc.sync.dma_start(
                    rv[h * 64:(h + 1) * 64, :],
                    a2a_out[g][h][:].rearrange("j d r -> d j r"))

        po_all = {}

        def proj_mm(g, oc, j, hh=None, first=False):
            """One pair of accumulation matmuls into po[g,oc]. hh=None
            contracts the full 128 rows; hh=0/1 contracts a 64-row half
            (so head-0 work can run while head-1's AllToAll is in
            flight — used for the final group)."""
            def go():
                if first:
                    # the final group's oc=1 accumulator borrows the idle
                    # QKV psum slot so both oc chains can be open at once
                    pool, tg = (psr, "pp") if (g == 3 and oc == 1) \
                        else (pop, "po")
                    po_all[g, oc] = pool.tile([128, 512], F32, tag=tg,
                                              name=f"po{g}_{oc}")
                po = po_all[g, oc]
                sl = slice(0, 128) if hh is None else \
                    slice(hh * 64, (hh + 1) * 64)
                rv = rv_all[g]
                for k, jj in enumerate((j, j + 1)):
                    nc.tensor.matmul(
                        po[:, 0:512],
                        rv[sl, jj * 128:(jj + 1) * 128],
                        wpt_sb[jj][sl, oc * 512:(oc + 1) * 512],
                        start=(first and k == 0), stop=False)
            return go

        def proj_fin(g, oc):
            b, qh = g // 2, g % 2
            def go():
                po = po_all[g, oc]
                # rank-1 bias broadcast as a final accumulation step
                nc.tensor.matmul(
                    po[:, 0:512], ones1[:],
                    bias_bf[:, oc * 512:(oc + 1) * 512],
                    start=False, stop=True)
                ot = dp.tile([128, 512], F32, tag="ot", name=f"ot{g}_{oc}")
                nc.vector.tensor_copy(ot[:], po[:, 0:512])
                nc.sync.dma_start(
                    OUT[b * RPB + qh * 128:b * RPB + qh * 128 + 128,
                        oc * 512:(oc + 1) * 512], ot[:])
            return go

        def proj_closures(g):
            for oc in range(2):
                for j in range(0, 8, 2):
                    yield proj_mm(g, oc, j, first=(j == 0))
                yield proj_fin(g, oc)

        def proj_h0_closures(g):
            for oc in range(2):
                for j in range(0, 8, 2):
                    yield proj_mm(g, oc, j, hh=0, first=(j == 0))

        def proj_h1_closures(g):
            for oc in range(2):
                for j in range(0, 8, 2):
                    yield proj_mm(g, oc, j, hh=1)
                yield proj_fin(g, oc)

        # ---------------- schedule ----------------
        # quarter 0 inline; PE idle anyway
        for cl in phase_a_qk_closures(0):
            cl()
        for cl in phase_a_v_closures(0):
            cl()
        bg.extend(phase_a_qk_closures(1))
        bg.extend(phase_a_v_closures(1))

        nf = attn_group(0, 0, 0, drain=3)
        phase_a_dmas(2, nc.scalar)
        bg.extend(phase_a_v_closures(2))
        nf = attn_group(0, 1, 0, drain=3,
                        inject={3: [nf, lambda: coll(0, 0)]})
        phase_a_dmas(3, nc.scalar)
        bg.extend(phase_a_v_closures(3))
        nf = attn_group(0, 0, 1, inject={3: [nf, lambda: coll(0, 1)]})
        load_wpt()
        bg.extend(phase_a_qk_closures(2))
        nf = attn_group(0, 1, 1, drain=3,
                        inject={3: [nf, lambda: coll(1, 0)]})
        bg.extend(phase_a_qk_closures(3))
        nf = attn_group(1, 0, 0, drain=3,
                        inject={3: [nf, lambda: coll(1, 1),
                                    lambda: proj_dma(0)]})
        bg.extend(proj_closures(0))
        nf = attn_group(1, 1, 0, drain=3,
                        inject={3: [nf, lambda: coll(2, 0),
                                    lambda: proj_dma(1)]})
        bg.extend(proj_closures(1))
        drain_bg(2)
        nf = attn_group(1, 0, 1, fexp=True,
                        inject={3: [nf, lambda: coll(2, 1),
                                    lambda: proj_dma(2)]})
        bg.extend(proj_closures(2))
        ph0 = list(proj_h0_closures(3))
        nf = attn_group(1, 1, 1, fexp=True, last=True, inject={
            3: [nf, lambda: coll(3, 0),
                lambda: proj_dma(3, heads=(0,))],
            11: ph0[0:2], 12: ph0[2:4], 13: ph0[4:6], 14: ph0[6:8]})
        nf()
        coll(3, 1)
        proj_dma(3, heads=(1,))
        drain_bg(len(bg))
        for cl in proj_h1_closures(3):
            cl()

    nc.compile()
    return nc


def prep_in_maps(X, Wq, Wk, Wv, Wp, bp):
    X = np.asarray(X, dtype=np.float32)
    Wq = np.asarray(Wq, dtype=np.float32)
    Wk = np.asarray(Wk, dtype=np.float32)
    Wv = np.asarray(Wv, dtype=np.float32)
    Wp = np.asarray(Wp, dtype=np.float32)
    bp = np.asarray(bp, dtype=np.float32)
    bf = ml_dtypes.bfloat16

    XT = X.reshape(N, C).T                                   # [C, N]
    xt8v = np.ascontiguousarray(
        XT.reshape(8, 128, N).transpose(1, 0, 2).reshape(128, 8 * N))
    xt8 = xt8v.astype(bf)
    xq8 = xt8v.astype(ml_dtypes.float8_e4m3fn)
    WPT = np.ascontiguousarray(Wp.T).astype(bf)              # [C, C]
    biasbf = np.ascontiguousarray(bp.reshape(1, C)).astype(bf)
    maskc = np.ascontiguousarray(
        (np.arange(128)[None, :] >= np.arange(128)[:, None])).astype(bf)
    zeros = np.zeros((64, N), dtype=bf)
    ident = np.ascontiguousarray(np.tile(np.eye(64), (2, 1))).astype(bf)

    def w3f(Wfull, i):
        Wc = Wfull[HPC * i:HPC * i + HPC].reshape(128, C)    # [m, c]
        WT = np.ascontiguousarray(Wc.T)                      # [c, m]
        return np.ascontiguousarray(
            WT.reshape(8, 128, 128).transpose(1, 0, 2).reshape(128, C))

    def w3(Wfull, i):
        return w3f(Wfull, i).astype(bf)

    in_maps = []
    for i in range(NCORES):
        in_maps.append({
            "xt8": xt8,
            "xq8": xq8,
            "wq8": (w3f(Wq, i) * 32.0).astype(ml_dtypes.float8_e4m3fn),
            "wk8": (w3f(Wk, i) * 32.0).astype(ml_dtypes.float8_e4m3fn),
            "wv3": w3(Wv, i),
            "wpt8": WPT,
            "biasbf": biasbf,
            "maskc": maskc,
            "zeros": zeros,
            "ident": ident,
        })
    return in_maps


def assemble(outs) -> np.ndarray:
    """outs[i]: [2*RPB, C]; core i owns rows [qh*1024+i*128,+128) per (b,qh)."""
    full = np.empty((N, C), dtype=np.float32)
    for i in range(NCORES):
        o = np.asarray(outs[i], dtype=np.float32)
        for b in range(B):
            for qh in range(2):
                full[b * T + qh * 1024 + i * 128:
                     b * T + qh * 1024 + (i + 1) * 128] = \
                    o[b * RPB + qh * 128:b * RPB + (qh + 1) * 128]
    return full.reshape(B, T, C)


def run(inputs, trace=False, trace_kwargs=None):
    if "nc" not in _CACHE:
        _CACHE["nc"] = build_nc()
    nc = _CACHE["nc"]
    in_maps = prep_in_maps(**inputs)
    res = run_bass_kernel_spmd(
        nc, in_maps, list(range(NCORES)), trace=trace,
        **(trace_kwargs or {}))
    out = assemble([res.results[i]["out"] for i in range(NCORES)])
    return out, res


def kernel(**inputs) -> np.ndarray:
    out, _ = run(inputs, trace=False)
    return out
